# revision 1
# baseline (speedup 1.0000x reference)
"""Trainium2 Bass kernel for nn_DistillLossContrastive.

Contract: kernel(**inputs) takes the FULL unsharded inputs and returns the
FULL output (a scalar f32 loss). Internally the BS axis is sharded across 8
NeuronCores (core b owns object b): each core computes its 32 pooled mask
features with a [32,8192]x[8192,512] float32r matmul chain over the streamed
point features; the per-object pooled block [513,32] is all-gathered once,
and every core redundantly computes the global 256x256 contrastive logits
matrix (fp16 operands, fp32 accumulate) + masked diagonal log-softmax losses
on device.
"""

import numpy as np

import concourse.bass as bass
import concourse.mybir as mybir
import concourse.tile as tile
from concourse import bacc
from concourse.bass_utils import run_bass_kernel_spmd
from concourse.masks import make_identity

BS, NPTS, NM, D = 8, 8192, 32, 512
NCORES = 8
NT = BS * NM          # 256 total masks
KT = NPTS // 128      # 64 contraction tiles of 128 points
DC = D // 128         # 4 chunks of the feature dim
# graduated net DMA sizes (in k-tiles): small first chunks to shorten the
# first-matmul latency, 2 MiB steady-state
DMA_KTS = [4, 4, 8, 8, 8, 8, 8, 8, 8]
assert sum(DMA_KTS) == KT

F32 = mybir.dt.float32
F32R = mybir.dt.float32r
F16 = mybir.dt.float16
AX_X = mybir.AxisListType.X
AX_C = mybir.AxisListType.C
ALU = mybir.AluOpType
ACT = mybir.ActivationFunctionType


def _build_program():
    nc = bacc.Bacc(
        "TRN2",
        target_bir_lowering=False,
        debug=False,
        num_devices=NCORES,
    )

    net = nc.dram_tensor("net", [len(DMA_KTS) * 128, 8 * D], F32, kind="ExternalInput")
    maskT = nc.dram_tensor("maskT", [128, KT * NM], F32, kind="ExternalInput")
    membT = nc.dram_tensor("membT", [D, NT], F32, kind="ExternalInput")
    lscale = nc.dram_tensor("lscale", [128, 1], F32, kind="ExternalInput")
    out = nc.dram_tensor("out", [1, 1], F32, kind="ExternalOutput")

    with tile.TileContext(nc) as tc:
        with (
            tc.tile_pool(name="consts", bufs=1) as consts,
            tc.tile_pool(name="netp", bufs=6) as netp,
            tc.tile_pool(name="maskp", bufs=1) as maskp,
            tc.tile_pool(name="work", bufs=1) as work,
            tc.tile_pool(name="ph2", bufs=1) as ph2,
            tc.tile_pool(name="ps", bufs=1, space="PSUM") as ps,
            tc.tile_pool(name="dramp", bufs=1, space="DRAM") as dramp,
        ):
            # ---- maskT in 4 chunks on the scalar ring (first chunk gates mm 0) ----
            maskT_sb = maskp.tile([128, KT * NM], F32R)
            MQ = KT * NM // 4
            for q in range(4):
                nc.scalar.dma_start(
                    maskT_sb[:, q * MQ : (q + 1) * MQ],
                    maskT[:, q * MQ : (q + 1) * MQ].bitcast(F32R),
                )

            # ---- phase 1: sum_feats = mask_pts @ net, one PSUM group ----
            ps_feats = ps.tile([NM, D], F32, name="ps_feats", tag="feats")
            k = 0
            for i, ksz in enumerate(DMA_KTS):
                nt = netp.tile([128, 8 * D], F32R, name="nt")
                nc.sync.dma_start(
                    nt[:, : ksz * D],
                    net[i * 128 : (i + 1) * 128, : ksz * D].bitcast(F32R),
                )
                for ks in range(ksz):
                    nc.tensor.matmul(
                        ps_feats[:, :],
                        maskT_sb[:, k * NM : (k + 1) * NM],
                        nt[:, ks * D : (ks + 1) * D],
                        start=(k == 0),
                        stop=(k == KT - 1),
                    )
                    k += 1

            # ---- small loads + constants (overlap the stream) ----
            mbT = []
            for c in range(DC):
                t = ph2.tile([128, NT], F32, name=f"mbTf{c}")
                nc.scalar.dma_start(t[:], membT[c * 128 : (c + 1) * 128, :])
                t16 = ph2.tile([128, NT], F16, name=f"mbT{c}")
                nc.vector.tensor_copy(t16[:], t[:])
                mbT.append(t16)
            ls_sb = consts.tile([128, 1], F32)
            nc.scalar.dma_start(ls_sb[:], lscale[:, :])
            es = consts.tile([128, 1], F32)
            nc.scalar.activation(es[:], ls_sb[:], ACT.Exp)
            # touch Ln early so its ACT table is resident before the tail
            ln_warm = consts.tile([1, 1], F32)
            nc.scalar.activation(ln_warm[:], es[0:1, :], ACT.Ln)
            ones_col = consts.tile([128, 1], F32)
            nc.vector.memset(ones_col[:], 1.0)
            id128 = consts.tile([128, 128], F32)
            make_identity(nc, id128[:])

            # ---- counts: DVE k-reduce + gpsimd partition-reduce; transpose the
            # [1,32] row to [32,1] with a zero-padded 32x32 DVE block transpose
            part = work.tile([128, NM], F32)
            nc.vector.reduce_sum(
                part[:],
                maskT_sb[:].bitcast(F32).rearrange("p (k m) -> p m k", k=KT),
                axis=AX_X,
            )
            cnt_row = work.tile([1, NM], F32)
            nc.gpsimd.tensor_reduce(cnt_row[:], part[:], axis=AX_C, op=ALU.add)
            cnt_sq = work.tile([NM, NM], F32)
            nc.vector.memset(cnt_sq[:], 0.0)
            nc.vector.tensor_copy(cnt_sq[0:1, :], cnt_row[:])
            cnt_sqT = work.tile([NM, NM], F32)
            nc.vector.transpose(cnt_sqT[:], cnt_sq[:])
            dn = work.tile([NM, 1], F32)
            nc.vector.tensor_scalar_add(dn[:], cnt_sqT[:, 0:1], 1e-12)
            rec = work.tile([NM, 1], F32)
            nc.vector.reciprocal(rec[:], dn[:])
            rec2 = work.tile([NM, 1], F32)
            nc.vector.tensor_mul(rec2[:], rec[:], es[0:NM, :])

            # ---- avg*es, transpose to [512, 32], single all-gather ----
            avg = work.tile([NM, D], F32)
            nc.vector.tensor_scalar_mul(avg[:], ps_feats[:], rec2[:])
            agin_sb = work.tile([128, DC * NM], F32)
            for cb in range(D // 32):
                nc.vector.transpose(
                    agin_sb[
                        32 * (cb % 4) : 32 * (cb % 4) + 32,
                        (cb // 4) * 32 : (cb // 4) * 32 + 32,
                    ],
                    avg[:, 32 * cb : 32 * cb + 32],
                )
            ag_in = dramp.tile([D, NM], F32)
            ag_out = dramp.tile([NCORES * D, NM], F32, addr_space="Shared")
            nc.scalar.dma_start(
                ag_in[0:D, :].rearrange("(c p) m -> p c m", p=128),
                agin_sb[:].rearrange("p (c m) -> p c m", m=NM),
            )
            nc.gpsimd.collective_compute(
                "AllGather",
                ALU.bypass,
                replica_groups=[list(range(NCORES))],
                ins=[ag_in[:].opt()],
                outs=[ag_out[:].opt()],
            )

            # ---- phase 2 (all cores redundantly): logits + losses ----
            agv = ag_out.rearrange("(b r) m -> b r m", b=NCORES)  # [8,512,32]
            ps_vs = ps.tile([1, NT], F32, name="ps_vs", tag="small", bufs=2)
            avT = []
            for c in range(DC):
                t = ph2.tile([128, NT], F32, name=f"avTf{c}", tag="sg", bufs=3)
                nc.sync.dma_start(
                    t[:].rearrange("p (b m) -> p b m", m=NM),
                    agv[:, c * 128 : (c + 1) * 128, :].rearrange("b p m -> p b m"),
                )
                t16 = ph2.tile([128, NT], F16, name=f"avT{c}")
                nc.vector.tensor_copy(t16[:], t[:])
                avT.append(t16)
                ab = ph2.tile([128, NT], F32, name="ab", tag="ab", bufs=2)
                nc.vector.tensor_mul(ab[:], t[:], t[:])
                nc.tensor.matmul(
                    ps_vs[:, :], ones_col[:], ab[:],
                    start=(c == 0), stop=(c == DC - 1),
                )
            vrow = ph2.tile([1, NT], F32, name="vrow")
            nc.vector.tensor_scalar(vrow[:], ps_vs[:], 0.0, None, op0=ALU.is_gt)
            one_sb = consts.tile([1, 1], F32)
            nc.vector.memset(one_sb[:], 1.0)
            vc = {}
            for r in range(2):
                pv = ps.tile([128, 1], F32, name=f"psv{r}", tag="small", bufs=2)
                nc.tensor.matmul(
                    pv[:, :], vrow[:, r * 128 : (r + 1) * 128], one_sb[:]
                )
                v = ph2.tile([128, 1], F32, name=f"vc{r}")
                nc.vector.tensor_copy(v[:], pv[:])
                vc[r] = v

            # logits chunks in PSUM (already scaled by exp(logit_scale))
            lg = {}
            for which, lhs_set, rhs_set in (("L", mbT, avT), ("T", avT, mbT)):
                for r in range(2):
                    ps_lg = ps.tile(
                        [128, NT], F32, name=f"pslg{which}{r}", tag="pslg", bufs=4
                    )
                    for c in range(DC):
                        nc.tensor.matmul(
                            ps_lg[:, :],
                            lhs_set[c][:, r * 128 : (r + 1) * 128],
                            rhs_set[c][:],
                            start=(c == 0),
                            stop=(c == DC - 1),
                        )
                    lg[(which, r)] = ps_lg

            # row-max (negated) for all four tiles, then batched Exp, then Ln
            mx = {}
            for key, t in lg.items():
                m = ph2.tile([128, 1], F32, name=f"mx{key[0]}{key[1]}")
                nc.vector.reduce_max(m[:], t[:], axis=AX_X, negate=True)
                mx[key] = m
            sm = {}
            for key, t in lg.items():
                ex = ph2.tile([128, NT], F32, name="ex", tag="ex", bufs=4)
                s = ph2.tile([128, 1], F32, name=f"sm{key[0]}{key[1]}")
                nc.scalar.activation(
                    ex[:], t[:], ACT.Exp, bias=mx[key][:], accum_out=s[:]
                )
                sm[key] = s
            lse = {}
            for key in lg:
                ln_t = ph2.tile([128, 1], F32, name=f"ln{key[0]}{key[1]}")
                nc.scalar.activation(ln_t[:], sm[key][:], ACT.Ln)
                l_t = ph2.tile([128, 1], F32, name=f"lse{key[0]}{key[1]}")
                nc.vector.tensor_sub(l_t[:], ln_t[:], mx[key][:])
                lse[key] = l_t

            ps_fin = ps.tile([1, 4], F32, name="ps_fin", tag="small", bufs=2)
            for r in range(2):
                dg = ph2.tile([128, 1], F32, name=f"dg{r}")
                dsc = ph2.tile([128, 128], F32, name="dsc", tag="dsc", bufs=2)
                nc.vector.tensor_mul(
                    dsc[:], lg[("L", r)][:, r * 128 : (r + 1) * 128], id128[:]
                )
                nc.vector.reduce_sum(dg[:], dsc[:], axis=AX_X)

                gt = ph2.tile([128, 1], F32, name=f"gt{r}")
                nc.vector.tensor_scalar(gt[:], vc[r][:], 0.0, None, op0=ALU.is_gt)

                stk = ph2.tile([128, 4], F32, name=f"stk{r}")
                for j, which in ((0, "L"), (2, "T")):
                    tmp = ph2.tile([128, 1], F32, name=f"tmp{which}{r}")
                    nc.vector.tensor_sub(tmp[:], lse[(which, r)][:], dg[:])
                    nc.vector.tensor_mul(stk[:, j : j + 1], tmp[:], gt[:])
                    nc.vector.tensor_scalar(
                        stk[:, j + 1 : j + 2], stk[:, j : j + 1], 0.0, None,
                        op0=ALU.is_gt,
                    )
                nc.tensor.matmul(
                    ps_fin[:, :], ones_col[:], stk[:], start=(r == 0), stop=(r == 1)
                )

            # ---- final scalar: mean over positive losses, both directions ----
            fin = ph2.tile([1, 4], F32, name="fin")
            nc.vector.tensor_copy(fin[:], ps_fin[:])
            res = ph2.tile([1, 5], F32, name="res")
            for j, (s_col, c_col) in enumerate(((0, 1), (2, 3))):
                nc.vector.tensor_scalar_max(
                    res[:, 0:1], fin[:, c_col : c_col + 1], 1.0
                )
                nc.vector.reciprocal(res[:, 1:2], res[:, 0:1])
                nc.vector.tensor_mul(
                    res[:, 2 + j : 3 + j], fin[:, s_col : s_col + 1], res[:, 1:2]
                )
                gtc = ph2.tile([1, 1], F32, name=f"gtc{j}")
                nc.vector.tensor_scalar(
                    gtc[:], fin[:, c_col : c_col + 1], 0.0, None, op0=ALU.is_gt
                )
                nc.vector.tensor_mul(
                    res[:, 2 + j : 3 + j], res[:, 2 + j : 3 + j], gtc[:]
                )
            nc.vector.tensor_add(res[:, 4:5], res[:, 2:3], res[:, 3:4])
            out_sb = ph2.tile([1, 1], F32, name="out_sb")
            nc.vector.tensor_scalar_mul(out_sb[:], res[:, 4:5], 0.5)
            nc.sync.dma_start(out[:, :], out_sb[:])

    nc.compile()
    return nc


_NC_CACHE = None


def _get_program():
    global _NC_CACHE
    if _NC_CACHE is None:
        _NC_CACHE = _build_program()
    return _NC_CACHE


def _make_in_maps(net_out, mask_embs, mask_pts, logit_scale):
    net_out = np.asarray(net_out, dtype=np.float32)
    mask_embs = np.asarray(mask_embs, dtype=np.float32)
    mask_pts = np.asarray(mask_pts, dtype=np.float32)
    membT = np.ascontiguousarray(mask_embs.T)
    ls = np.broadcast_to(
        np.array(logit_scale, dtype=np.float32).reshape(1, 1), (128, 1)
    ).copy()
    in_maps = []
    for b in range(NCORES):
        # pre-tile so each DMA chunk is a contiguous [128, ksz*D] block
        nb = net_out[b * NPTS : (b + 1) * NPTS].reshape(KT, 128, D)
        net_b = np.zeros((len(DMA_KTS) * 128, 8 * D), np.float32)
        k = 0
        for i, ksz in enumerate(DMA_KTS):
            net_b[i * 128 : (i + 1) * 128, : ksz * D] = (
                nb[k : k + ksz].transpose(1, 0, 2).reshape(128, ksz * D)
            )
            k += ksz
        m = mask_pts[b]  # [NM, NPTS]
        maskT_b = np.ascontiguousarray(
            m.reshape(NM, KT, 128).transpose(2, 1, 0).reshape(128, KT * NM)
        )
        in_maps.append(
            {"net": net_b, "maskT": maskT_b, "membT": membT, "lscale": ls}
        )
    return in_maps


def _install_ntff_shim():
    """Provide the antenv.axon_hooks registry this image lacks and register
    the ctypes NTFF hook (same as trn_agent_boot would)."""
    import sys
    import types

    if "antenv.axon_hooks" not in sys.modules:
        import antenv

        mod = types.ModuleType("antenv.axon_hooks")
        holder = [None]
        mod.set_axon_ntff_profile_hook = lambda h: holder.__setitem__(0, h)
        mod.get_axon_ntff_profile_hook = lambda: holder[0]
        sys.modules["antenv.axon_hooks"] = mod
        antenv.axon_hooks = mod
    from antenv.axon_hooks import (
        get_axon_ntff_profile_hook,
        set_axon_ntff_profile_hook,
    )

    if get_axon_ntff_profile_hook() is None:
        from trn_agent_boot.trn_boot import _ntff_profile_via_ctypes

        set_axon_ntff_profile_hook(
            _ntff_profile_via_ctypes("/opt/axon/libaxon_pjrt.so")
        )


def run(net_out, mask_embs, mask_pts, logit_scale, pt_offset=None, trace=False):
    """Run on 8 NeuronCores; returns (scalar ndarray, BassKernelResults)."""
    if trace:
        try:
            _install_ntff_shim()
        except Exception:
            pass
    nc = _get_program()
    in_maps = _make_in_maps(net_out, mask_embs, mask_pts, logit_scale)
    r = run_bass_kernel_spmd(nc, in_maps, core_ids=list(range(NCORES)), trace=trace)
    val = np.asarray(r.results[0]["out"], dtype=np.float32).reshape(())
    return val, r


def kernel(net_out, mask_embs, mask_pts, logit_scale, pt_offset=None):
    val, _ = run(net_out, mask_embs, mask_pts, logit_scale, pt_offset)
    return val



# revision 4
# speedup vs baseline: 1.2751x; 1.2751x over previous
"""Trainium2 Bass kernel for nn_DistillLossContrastive.

Contract: kernel(**inputs) takes the FULL unsharded inputs and returns the
FULL output (a scalar f32 loss). The BS axis is sharded across 8 NeuronCores
(core b owns object b). Per core:

  phase 1: stream the 8192x512 point block (fp16) and accumulate the 32
    pooled mask features with a [32,8192]x[8192,512] matmul chain; counts
    come from a DVE reduce of the mask overlapped with the stream.
  phase 2 (local): transpose pooled feats via PE, compute the LOCAL column
    block of the global logits matrix lbT[j,i] = l_ij for this core's 32
    masks j against all 256 mask_embs i, then exp + row/col partial sums.
  exchange: a tiny [128,10] f32 packet per core (row-softmax partials R,
    scattered diag/valid/pts entries) is summed across cores - everything
    in the packet is sum-combinable, so an 8-way gather+add equals the
    global result. Transport is either the gpsimd AllGather collective or
    direct SBUF->SBUF remote DMA writes (XOR-slot convention).
  final: every core redundantly computes the masked diagonal log-softmax
    losses and the nonzero-mean scalar.
"""

import numpy as np

import concourse.bass as bass
import concourse.mybir as mybir
import concourse.tile as tile
from concourse import bacc
from concourse.bass_utils import run_bass_kernel_spmd

BS, NPTS, NM, D = 8, 8192, 32, 512
NCORES = 8
NT = BS * NM          # 256 total masks
KT = NPTS // 128      # 64 contraction tiles of 128 points
DC = D // 128         # 4 chunks of the feature dim
PKW = 10              # packet width: [R0 R1 | D V L G]x2 halves

STREAM = "f16"        # "f16" | "f8"
TRANSPORT = "cc"      # "cc" | "rdma"

DMA_KTS = [2, 2, 4, 8, 8, 8, 8, 8, 8, 8]
assert sum(DMA_KTS) == KT

F32 = mybir.dt.float32
F16 = mybir.dt.float16
F8 = mybir.dt.float8e4
AX_X = mybir.AxisListType.X
ALU = mybir.AluOpType
ACT = mybir.ActivationFunctionType


def _build_program():
    sd = F16 if STREAM == "f16" else F8
    nc = bacc.Bacc(
        "TRN2",
        target_bir_lowering=False,
        debug=False,
        num_devices=NCORES,
    )

    net = nc.dram_tensor("net", [len(DMA_KTS) * 128, 8 * D], sd, kind="ExternalInput")
    maskT = nc.dram_tensor("maskT", [128, KT * NM], sd, kind="ExternalInput")
    membT = nc.dram_tensor("membT", [D, NT], F16, kind="ExternalInput")
    psel = nc.dram_tensor("psel", [NM, NT], F32, kind="ExternalInput")
    id32 = nc.dram_tensor("id32", [NM, NM], F16, kind="ExternalInput")
    lscale = nc.dram_tensor("lscale", [128, 1], F32, kind="ExternalInput")
    out = nc.dram_tensor("out", [1, 1], F32, kind="ExternalOutput")

    with tile.TileContext(nc) as tc:
        with (
            tc.tile_pool(name="consts", bufs=1) as consts,
            tc.tile_pool(name="netp", bufs=6) as netp,
            tc.tile_pool(name="maskp", bufs=1) as maskp,
            tc.tile_pool(name="work", bufs=1) as work,
            tc.tile_pool(name="ps", bufs=1, space="PSUM") as ps,
            tc.tile_pool(name="dramp", bufs=1, space="DRAM") as dramp,
        ):
            # ---- maskT in 4 chunks on the scalar ring (first chunk gates mm 0) ----
            maskT_sb = maskp.tile([128, KT * NM], sd)
            MQ = KT * NM // 4
            for q in range(4):
                nc.scalar.dma_start(
                    maskT_sb[:, q * MQ : (q + 1) * MQ],
                    maskT[:, q * MQ : (q + 1) * MQ],
                )

            # ---- rdma transport: prep the 7 peer writes early (trigger later) ----
            if TRANSPORT == "rdma":
                pkt = work.tile([128, PKW], F32, name="pkt")
                recv = work.tile([128, NCORES * PKW], F32, name="recv")
                rsem = nc.alloc_semaphore("rsem")
                lsem = nc.alloc_semaphore("lsem")
                for d in range(1, NCORES):
                    rdests = [None] * NCORES
                    rdests[d] = (0, d)
                    nc.gpsimd.remote_dma_broadcast(
                        recv[:, d * PKW : (d + 1) * PKW],
                        pkt[:, :],
                        rsem,
                        lsem,
                        rdests=rdests,
                    )

            # ---- phase 1: sum_feats = mask_pts @ net, one PSUM group ----
            ps_feats = ps.tile([NM, D], F32, name="ps_feats", tag="feats")
            k = 0
            for i, ksz in enumerate(DMA_KTS):
                nt = netp.tile([128, 8 * D], sd, name="nt")
                nc.sync.dma_start(
                    nt[:, : ksz * D],
                    net[i * 128 : (i + 1) * 128, : ksz * D],
                )
                if STREAM == "f16":
                    for ks in range(ksz):
                        nc.tensor.matmul(
                            ps_feats[:, :],
                            maskT_sb[:, k * NM : (k + 1) * NM],
                            nt[:, ks * D : (ks + 1) * D],
                            start=(k == 0),
                            stop=(k == KT - 1),
                        )
                        k += 1
                else:
                    for ks in range(ksz // 2):
                        nc.tensor.matmul(
                            ps_feats[:, :],
                            maskT_sb[:, k * NM : (k + 2) * NM].rearrange(
                                "p (t m) -> p t m", t=2
                            ),
                            nt[:, 2 * ks * D : (2 * ks + 2) * D].rearrange(
                                "p (t n) -> p t n", t=2
                            ),
                            start=(k == 0),
                            stop=(k == KT - 2),
                            perf_mode=mybir.MatmulPerfMode.DoubleRow,
                        )
                        k += 2

            # ---- small loads (overlap the stream, scalar ring) ----
            mbT = []
            for c in range(DC):
                t = consts.tile([128, NT], F16, name=f"mbT{c}")
                nc.scalar.dma_start(t[:], membT[c * 128 : (c + 1) * 128, :])
                mbT.append(t)
            psel_sb = consts.tile([NM, NT], F32, name="psel_sb")
            nc.scalar.dma_start(psel_sb[:], psel[:, :])
            id32_sb = consts.tile([NM, NM], F16, name="id32_sb")
            nc.scalar.dma_start(id32_sb[:], id32[:, :])
            ls_sb = consts.tile([128, 1], F32)
            nc.scalar.dma_start(ls_sb[:], lscale[:, :])
            es = consts.tile([128, 1], F32)
            nc.scalar.activation(es[:], ls_sb[:], ACT.Exp)
            # touch Ln early so its ACT table is resident before the tail
            ln_warm = consts.tile([1, 1], F32)
            nc.scalar.activation(ln_warm[:], es[0:1, :], ACT.Ln)
            ones128 = consts.tile([128, 1], F32)
            nc.vector.memset(ones128[:], 1.0)
            ones32 = consts.tile([NM, 1], F32)
            nc.vector.memset(ones32[:], 1.0)

            # ---- counts (during stream): DVE k-reduce + ones-matmul ----
            if STREAM == "f16":
                mask_cnt_src = maskT_sb
            else:
                mask_cnt_src = work.tile([128, KT * NM], F16, name="mask16")
                nc.vector.tensor_copy(mask_cnt_src[:], maskT_sb[:])
            part = work.tile([128, NM], F32)
            nc.vector.reduce_sum(
                part[:],
                mask_cnt_src[:].rearrange("p (k m) -> p m k", k=KT),
                axis=AX_X,
            )
            cnt_ps = ps.tile([1, NM], F32, name="cnt_ps", tag="small", bufs=2)
            nc.tensor.matmul(cnt_ps[:, :], ones128[:], part[:])
            cnt_sq = work.tile([NM, NM], F32)
            nc.vector.memset(cnt_sq[:], 0.0)
            nc.vector.tensor_copy(cnt_sq[0:1, :], cnt_ps[:, :])
            cnt_sqT = work.tile([NM, NM], F32)
            nc.vector.transpose(cnt_sqT[:], cnt_sq[:])
            dn = work.tile([NM, 1], F32)
            nc.vector.tensor_scalar_add(dn[:], cnt_sqT[:, 0:1], 1e-12)
            rec = work.tile([NM, 1], F32)
            nc.vector.reciprocal(rec[:], dn[:])
            rec2es = work.tile([NM, 1], F32)
            nc.vector.tensor_mul(rec2es[:], rec[:], es[0:NM, :])
            valid = work.tile([NM, 1], F32)
            nc.vector.tensor_scalar(
                valid[:], cnt_sqT[:, 0:1], 0.0, None, op0=ALU.is_gt
            )

            # ---- phase 2 (local): feats -> avT -> local logits block ----
            feats16 = work.tile([NM, D], F16)
            nc.vector.tensor_copy(feats16[:], ps_feats[:])
            avT_ps = ps.tile([128, 128], F16, name="avT_ps", tag="avt")
            for c in range(DC):
                nc.tensor.transpose(
                    avT_ps[:, c * NM : (c + 1) * NM],
                    feats16[:, c * 128 : (c + 1) * 128],
                    id32_sb[:],
                )
            avT16 = work.tile([128, 128], F16)
            nc.vector.tensor_copy(avT16[:], avT_ps[:])
            ps_lbT = ps.tile([NM, NT], F32, name="ps_lbT", tag="lbt")
            for c in range(DC):
                nc.tensor.matmul(
                    ps_lbT[:, :],
                    avT16[:, c * NM : (c + 1) * NM],
                    mbT[c][:],
                    start=(c == 0),
                    stop=(c == DC - 1),
                )

            # diag of the scaled block: rec2es_j * sum_i(psel o raw)
            dtmp = work.tile([NM, NT], F32, name="dtmp")
            nc.vector.tensor_mul(dtmp[:], ps_lbT[:], psel_sb[:])
            dr = work.tile([NM, 1], F32)
            nc.vector.reduce_sum(dr[:], dtmp[:], axis=AX_X)
            diag = work.tile([NM, 1], F32)
            nc.vector.tensor_mul(diag[:], dr[:], rec2es[:])

            # exp of scaled block + pts denominators (row sums)
            expb = work.tile([NM, NT], F32, name="expb")
            pden = work.tile([NM, 1], F32)
            nc.scalar.activation(
                expb[:], ps_lbT[:], ACT.Exp, scale=rec2es[:], accum_out=pden[:]
            )

            # pts losses for our masks
            lnden = work.tile([NM, 1], F32)
            nc.scalar.activation(lnden[:], pden[:], ACT.Ln)
            rhs4 = work.tile([NM, 4], F32, name="rhs4")
            nc.vector.tensor_mul(rhs4[:, 0:1], diag[:], valid[:])
            nc.vector.tensor_copy(rhs4[:, 1:2], valid[:])
            pl = work.tile([NM, 1], F32)
            nc.vector.tensor_sub(pl[:], lnden[:], diag[:])
            nc.vector.tensor_mul(rhs4[:, 2:3], pl[:], valid[:])
            nc.vector.tensor_scalar(
                rhs4[:, 3:4], rhs4[:, 2:3], 0.0, None, op0=ALU.is_gt
            )

            # R partials (texts row sums over local j) + scatter into i-space
            if TRANSPORT != "rdma":
                pkt = work.tile([128, PKW], F32, name="pkt")
            for c in range(2):
                r_ps = ps.tile([128, 1], F32, name=f"r_ps{c}", tag="small", bufs=2)
                nc.tensor.matmul(
                    r_ps[:, :], expb[:, c * 128 : (c + 1) * 128], ones32[:]
                )
                nc.vector.tensor_copy(pkt[:, c : c + 1], r_ps[:, :])
                sc_ps = ps.tile([128, 4], F32, name=f"sc_ps{c}", tag="small", bufs=2)
                nc.tensor.matmul(
                    sc_ps[:, :], psel_sb[:, c * 128 : (c + 1) * 128], rhs4[:]
                )
                for q in range(4):
                    nc.vector.tensor_copy(
                        pkt[:, 2 + 2 * q + c : 3 + 2 * q + c], sc_ps[:, q : q + 1]
                    )

            # ---- exchange: 8-way sum of packets ----
            summed = work.tile([128, PKW], F32, name="summed")
            if TRANSPORT == "cc":
                ag_in = dramp.tile([PKW * 128], F32)
                ag_out = dramp.tile([NCORES, PKW * 128], F32, addr_space="Shared")
                nc.scalar.dma_start(
                    ag_in[:].rearrange("(w p) -> p w", p=128), pkt[:, :]
                )
                nc.gpsimd.collective_compute(
                    "AllGather",
                    ALU.bypass,
                    replica_groups=[list(range(NCORES))],
                    ins=[ag_in[:].opt()],
                    outs=[ag_out[:].opt()],
                )
                gath = work.tile([128, NCORES * PKW], F32, name="gath")
                nc.sync.dma_start(
                    gath[:].rearrange("p (s w) -> p s w", s=NCORES),
                    ag_out[:, :].rearrange("s (w p) -> p s w", p=128),
                )
                nc.vector.reduce_sum(
                    summed[:],
                    gath[:].rearrange("p (s w) -> p w s", w=PKW),
                    axis=AX_X,
                )
            else:
                nc.gpsimd.bir_kernel_barrier_wait([list(range(NCORES))])
                nc.gpsimd.trigger_dma(count=None)
                nc.vector.tensor_copy(recv[:, 0:PKW], pkt[:, :])
                nc.vector.wait_ge(rsem, 14)
                nc.vector.reduce_sum(
                    summed[:],
                    recv[:].rearrange("p (s w) -> p w s", w=PKW),
                    axis=AX_X,
                )

            # ---- final scalar (redundant on every core) ----
            # summed cols: 0,1 = R halves; 2+2q+c = quantity q (D,V,L,G), half c
            lnS = work.tile([128, 2], F32)
            nc.scalar.activation(lnS[:], summed[:, 0:2], ACT.Ln)
            fin8 = work.tile([128, 8], F32, name="fin8")
            t1 = work.tile([128, 2], F32)
            for c in range(2):
                nc.vector.tensor_mul(
                    t1[:, c : c + 1], lnS[:, c : c + 1], summed[:, 4 + c : 5 + c]
                )
                nc.vector.tensor_sub(
                    fin8[:, c : c + 1], t1[:, c : c + 1], summed[:, 2 + c : 3 + c]
                )
                nc.vector.tensor_scalar(
                    fin8[:, 2 + c : 3 + c], fin8[:, c : c + 1], 0.0, None,
                    op0=ALU.is_gt,
                )
                nc.vector.tensor_copy(fin8[:, 4 + c : 5 + c], summed[:, 6 + c : 7 + c])
                nc.vector.tensor_copy(fin8[:, 6 + c : 7 + c], summed[:, 8 + c : 9 + c])
            ps_fin = ps.tile([1, 8], F32, name="ps_fin", tag="small", bufs=2)
            nc.tensor.matmul(ps_fin[:, :], ones128[:], fin8[:])

            f = work.tile([1, 8], F32)
            nc.vector.tensor_copy(f[:], ps_fin[:])
            r4 = work.tile([1, 4], F32)
            nc.vector.tensor_add(r4[:, 0:1], f[:, 0:1], f[:, 1:2])  # texts sum
            nc.vector.tensor_add(r4[:, 1:2], f[:, 2:3], f[:, 3:4])  # texts cnt
            nc.vector.tensor_add(r4[:, 2:3], f[:, 4:5], f[:, 5:6])  # pts sum
            nc.vector.tensor_add(r4[:, 3:4], f[:, 6:7], f[:, 7:8])  # pts cnt
            res = work.tile([1, 6], F32)
            for j, (s_col, c_col) in enumerate(((0, 1), (2, 3))):
                nc.vector.tensor_scalar_max(
                    res[:, 0:1], r4[:, c_col : c_col + 1], 1.0
                )
                nc.vector.reciprocal(res[:, 1:2], res[:, 0:1])
                nc.vector.tensor_mul(
                    res[:, 2 + j : 3 + j], r4[:, s_col : s_col + 1], res[:, 1:2]
                )
                gtc = work.tile([1, 1], F32, name=f"gtc{j}")
                nc.vector.tensor_scalar(
                    gtc[:], r4[:, c_col : c_col + 1], 0.0, None, op0=ALU.is_gt
                )
                nc.vector.tensor_mul(
                    res[:, 2 + j : 3 + j], res[:, 2 + j : 3 + j], gtc[:]
                )
            nc.vector.tensor_add(res[:, 4:5], res[:, 2:3], res[:, 3:4])
            out_sb = work.tile([1, 1], F32, name="out_sb")
            nc.vector.tensor_scalar_mul(out_sb[:], res[:, 4:5], 0.5)
            nc.sync.dma_start(out[:, :], out_sb[:])

    nc.compile()
    return nc


_NC_CACHE = None


def _get_program():
    global _NC_CACHE
    if _NC_CACHE is None:
        _NC_CACHE = _build_program()
    return _NC_CACHE


def _stream_np_dtype():
    if STREAM == "f16":
        return np.float16
    import ml_dtypes

    return ml_dtypes.float8_e4m3fn


def _make_in_maps(net_out, mask_embs, mask_pts, logit_scale):
    sdt = _stream_np_dtype()
    net_out = np.asarray(net_out, dtype=np.float32)
    mask_embs = np.asarray(mask_embs, dtype=np.float32)
    mask_pts = np.asarray(mask_pts, dtype=np.float32)
    membT = np.ascontiguousarray(mask_embs.T).astype(np.float16)
    id32 = np.eye(NM, dtype=np.float16)
    ls = np.broadcast_to(
        np.array(logit_scale, dtype=np.float32).reshape(1, 1), (128, 1)
    ).copy()
    in_maps = []
    for b in range(NCORES):
        # pre-tile so each DMA chunk is a contiguous [128, ksz*D] block
        nb = net_out[b * NPTS : (b + 1) * NPTS].reshape(KT, 128, D).astype(sdt)
        net_b = np.zeros((len(DMA_KTS) * 128, 8 * D), sdt)
        k = 0
        for i, ksz in enumerate(DMA_KTS):
            net_b[i * 128 : (i + 1) * 128, : ksz * D] = (
                nb[k : k + ksz].transpose(1, 0, 2).reshape(128, ksz * D)
            )
            k += ksz
        m = mask_pts[b]  # [NM, NPTS]
        maskT_b = np.ascontiguousarray(
            m.reshape(NM, KT, 128).transpose(2, 1, 0).reshape(128, KT * NM)
        ).astype(sdt)
        psel_b = np.zeros((NM, NT), np.float32)
        psel_b[np.arange(NM), b * NM + np.arange(NM)] = 1.0
        in_maps.append(
            {
                "net": net_b,
                "maskT": maskT_b,
                "membT": membT,
                "psel": psel_b,
                "id32": id32,
                "lscale": ls,
            }
        )
    return in_maps


def _install_ntff_shim():
    """Provide the antenv.axon_hooks registry this image lacks and register
    the ctypes NTFF hook (same as trn_agent_boot would)."""
    import sys
    import types

    if "antenv.axon_hooks" not in sys.modules:
        import antenv

        mod = types.ModuleType("antenv.axon_hooks")
        holder = [None]
        mod.set_axon_ntff_profile_hook = lambda h: holder.__setitem__(0, h)
        mod.get_axon_ntff_profile_hook = lambda: holder[0]
        sys.modules["antenv.axon_hooks"] = mod
        antenv.axon_hooks = mod
    from antenv.axon_hooks import (
        get_axon_ntff_profile_hook,
        set_axon_ntff_profile_hook,
    )

    if get_axon_ntff_profile_hook() is None:
        from trn_agent_boot.trn_boot import _ntff_profile_via_ctypes

        set_axon_ntff_profile_hook(
            _ntff_profile_via_ctypes("/opt/axon/libaxon_pjrt.so")
        )


def run(net_out, mask_embs, mask_pts, logit_scale, pt_offset=None, trace=False):
    """Run on 8 NeuronCores; returns (scalar ndarray, BassKernelResults)."""
    if trace:
        try:
            _install_ntff_shim()
        except Exception:
            pass
    nc = _get_program()
    in_maps = _make_in_maps(net_out, mask_embs, mask_pts, logit_scale)
    r = run_bass_kernel_spmd(nc, in_maps, core_ids=list(range(NCORES)), trace=trace)
    val = np.asarray(r.results[0]["out"], dtype=np.float32).reshape(())
    return val, r


def kernel(net_out, mask_embs, mask_pts, logit_scale, pt_offset=None):
    val, _ = run(net_out, mask_embs, mask_pts, logit_scale, pt_offset)
    return val
